# revision 9
# baseline (speedup 1.0000x reference)
"""LoftQ linear (4-bit blockwise dequant + linear + LoRA) on 8 trn2 cores.

out = x @ W^T + bias + 2.0 * (x @ A^T) @ B^T
  W[o,i] = (idx[o,i] * 2/15 - 1) * scales[o, i//64]   (idx = 4-bit nibbles)

Sharding: column-parallel — qweight/scales/bias/lora_B sharded along
out_features (4096 -> 512 per core); x and lora_A replicated; outputs
concatenated on host.

Device kernel (per core), all layouts prepared host-side:
  - contraction axis i is permuted to i' = [even i, odd i] so the nibble
    unpack of host-pre-transposed packed bytes lands in contiguous
    partition-tile halves (no on-chip transposes at all).
  - inputs are host-packed into [128, nblk, 512] form so each tensor loads
    with one (or few) large dma_start; DMA work is split across the sync
    HWDGE ring (weights), gpsimd SWDGE (x), and scalar HWDGE (outputs).
  - dequant: bitwise unpack (DVE) -> affine c*v-1 (ScalarE, fp16)
    -> *scale fp16 (DVE) -> + (2BA)^T bf16 (DVE; rank-16 lora product is
    host-precomputed weight preprocessing)
  - main: 512 bf16 matmuls [K=128,M=128,N=512], psum accumulate over i',
    bias added in the psum->sbuf copy (ScalarE), out dma on scalar ring.
"""

import numpy as np
import ml_dtypes

OUT_F = 4096
IN_F = 4096
T = 2048  # 2*1024 tokens
R = 16
NCORES = 8
O_SH = OUT_F // NCORES  # 512
IPH = IN_F // 2  # 2048 packed byte-rows
C16 = 2.0 / 15.0
NQ = IPH // 128  # 16 packed tiles
NI = IN_F // 128  # 32 i' chunks
NO = O_SH // 128  # 4 o tiles
NT = T // 512  # 4 t chunks
NBA = 4  # ba DMA chunks

BF16 = ml_dtypes.bfloat16
FP16 = np.float16

_cached = {}


def _build_nc():
    import concourse.bacc as bacc
    import concourse.mybir as mybir
    from concourse.tile import TileContext

    f32 = mybir.dt.float32
    bf16 = mybir.dt.bfloat16
    fp16 = mybir.dt.float16
    u8 = mybir.dt.uint8
    AF = mybir.ActivationFunctionType
    OP = mybir.AluOpType

    nc = bacc.Bacc("TRN2", target_bir_lowering=False)

    xt = nc.dram_tensor("xt", [128, NT, NI, 512], bf16, kind="ExternalInput")
    qwt = nc.dram_tensor("qwt", [128, NQ, O_SH], u8, kind="ExternalInput")
    st = nc.dram_tensor("st", [128, NQ, O_SH], fp16, kind="ExternalInput")
    ba = nc.dram_tensor("ba", [128, NI, O_SH], bf16, kind="ExternalInput")
    bias = nc.dram_tensor("bias", [O_SH, 1], f32, kind="ExternalInput")
    out = nc.dram_tensor("out", [O_SH, T], f32, kind="ExternalOutput")

    with TileContext(nc) as tc:
        with (
            tc.tile_pool(name="w", bufs=1) as wpool,
            tc.tile_pool(name="x", bufs=2) as xpool,
            tc.tile_pool(name="cst", bufs=1) as cpool,
            tc.tile_pool(name="dq", bufs=2) as dqpool,
            tc.tile_pool(name="outp", bufs=3) as opool,
            tc.tile_pool(name="ps", bufs=3, space="PSUM") as pspool,
        ):
            # big resident inputs — one large DMA each on the sync ring
            qb = cpool.tile([128, NQ, O_SH], u8, tag="qb", name="qb")
            nc.sync.dma_start(out=qb[:], in_=qwt[:])
            stb = cpool.tile([128, NQ, O_SH], fp16, tag="stb", name="stb")
            nc.sync.dma_start(out=stb[:], in_=st[:])
            bab = []
            nba = NI // NBA  # tiles per ba chunk
            for m in range(NBA):
                bt_m = cpool.tile([128, nba, O_SH], bf16, tag=f"ba{m}", name=f"bab{m}")
                nc.scalar.dma_start(out=bt_m[:], in_=ba[:, m * nba : (m + 1) * nba, :])
                bab.append(bt_m)
            bias_sb = []
            for ot in range(NO):
                btile = cpool.tile([128, 1], f32, tag=f"bias{ot}", name=f"biassb{ot}")
                nc.sync.dma_start(out=btile[:], in_=bias[ot * 128 : (ot + 1) * 128, :])
                bias_sb.append(btile)

            # persistent dequantized weight tiles W_eff^T: NI x [128 i', O_SH]
            W = [
                wpool.tile([128, O_SH], bf16, tag=f"w{j}", name=f"wt{j}")
                for j in range(NI)
            ]

            def ba_slice(j):
                return bab[j // nba][:, j % nba, :]

            # first x chunk early so PE can start as soon as W tiles appear
            xcs = {}
            xcs[0] = xpool.tile([128, NI, 512], bf16, tag="xc", name="xc0")
            nc.gpsimd.dma_start(out=xcs[0][:], in_=xt[:, 0])

            # dequant: packed tile k -> W[k] (lo nibbles) and W[NQ+k] (hi)
            # engine split: DVE unpack+hi-affine+mults, ScalarE lo-affine,
            # GpSimd the lora adds
            for k in range(NQ):
                lo = dqpool.tile([128, O_SH], u8, tag="lo", name=f"lo{k}")
                nc.vector.tensor_scalar(lo[:], qb[:, k, :], 15, None, OP.bitwise_and)
                hi = dqpool.tile([128, O_SH], u8, tag="hi", name=f"hi{k}")
                nc.vector.tensor_scalar(
                    hi[:], qb[:, k, :], 4, 15, OP.logical_shift_right, OP.bitwise_and
                )
                ulo = dqpool.tile([128, O_SH], fp16, tag="ulo", name=f"ulo{k}")
                nc.scalar.activation(ulo[:], lo[:], AF.Copy, bias=-1.0, scale=C16)
                uhi = dqpool.tile([128, O_SH], fp16, tag="uhi", name=f"uhi{k}")
                nc.vector.tensor_scalar(uhi[:], hi[:], C16, -1.0, OP.mult, OP.add)
                nc.vector.tensor_tensor(W[k][:], ulo[:], stb[:, k, :], OP.mult)
                nc.vector.tensor_tensor(W[NQ + k][:], uhi[:], stb[:, k, :], OP.mult)
                nc.gpsimd.tensor_tensor(W[k][:], W[k][:], ba_slice(k), OP.add)
                nc.gpsimd.tensor_tensor(
                    W[NQ + k][:], W[NQ + k][:], ba_slice(NQ + k), OP.add
                )

            # accumulate in W-pair production order so PE can chase dequant
            ICORD = [m for k in range(NQ) for m in (k, NQ + k)]

            # main matmul, streaming x by t-chunk (one big SWDGE DMA per chunk)
            for tcn in range(NT):
                if tcn not in xcs:
                    xcs[tcn] = xpool.tile(
                        [128, NI, 512], bf16, tag="xc", name=f"xc{tcn}"
                    )
                    nc.gpsimd.dma_start(out=xcs[tcn][:], in_=xt[:, tcn])
                xc = xcs[tcn]
                for ot in range(NO):
                    p = pspool.tile([128, 512], f32, tag="mm", name=f"p{tcn}_{ot}")
                    for n, ic in enumerate(ICORD):
                        nc.tensor.matmul(
                            p[:],
                            W[ic][:, ot * 128 : (ot + 1) * 128],
                            xc[:, ic, :],
                            start=(n == 0),
                            stop=(n == NI - 1),
                        )
                    o_sb = opool.tile([128, 512], f32, tag="osb", name=f"osb{tcn}_{ot}")
                    nc.scalar.activation(
                        o_sb[:], p[:], AF.Identity, bias=bias_sb[ot][:], scale=1.0
                    )
                    nc.scalar.dma_start(
                        out=out[ot * 128 : (ot + 1) * 128, tcn * 512 : (tcn + 1) * 512],
                        in_=o_sb[:],
                    )
    nc.compile()
    return nc


def _pack_rows(a, nblk):
    """[nblk*128, F] -> [128, nblk, F] with blk j, partition p = row j*128+p."""
    f = a.shape[1]
    return np.ascontiguousarray(a.reshape(nblk, 128, f).transpose(1, 0, 2))


def prep_inputs(x, qweight, scales, bias, lora_A, lora_B):
    """Host-side layout prep + sharding. Returns per-core input maps."""
    x2d = np.ascontiguousarray(x.reshape(T, IN_F))
    xt = x2d.T  # [IN_F, T]
    # i' permutation: even original i first, then odd
    xp = np.concatenate([xt[0::2], xt[1::2]], axis=0)
    xb = _pack_rows(xp, NI)  # [128, NI, T]
    xb = np.ascontiguousarray(
        xb.reshape(128, NI, NT, 512).transpose(0, 2, 1, 3)
    ).astype(BF16)  # [128, NT, NI, 512]

    ap = np.ascontiguousarray(
        np.concatenate([lora_A[:, 0::2], lora_A[:, 1::2]], axis=1)
    ).astype(np.float32)  # [R, IN_F] permuted

    qw2 = qweight.reshape(OUT_F, IPH)  # byte (o, ip) holds i=2ip (lo), 2ip+1 (hi)
    sc2 = scales.reshape(OUT_F, IN_F // 64)

    in_maps = []
    for c in range(NCORES):
        o0, o1 = c * O_SH, (c + 1) * O_SH
        qwt_c = _pack_rows(qw2[o0:o1].T, NQ).astype(np.uint8)  # [128, NQ, O_SH]
        # scale for (ip, o) = scales[o, ip//32] (same for lo and hi nibble)
        st_c = _pack_rows(np.repeat(sc2[o0:o1].T, 32, axis=0), NQ).astype(FP16)
        ba_c = _pack_rows(
            (ap.T @ (2.0 * lora_B[o0:o1].T)).astype(np.float32), NI
        ).astype(BF16)  # [128, NI, O_SH]
        bias_c = np.ascontiguousarray(bias[o0:o1].reshape(O_SH, 1)).astype(np.float32)
        in_maps.append(
            {"xt": xb, "qwt": qwt_c, "st": st_c, "ba": ba_c, "bias": bias_c}
        )
    return in_maps


def run(in_maps, trace=False):
    from concourse import bass_utils

    if "nc" not in _cached:
        _cached["nc"] = _build_nc()
    res = bass_utils.run_bass_kernel_spmd(
        _cached["nc"], in_maps, list(range(NCORES)), trace=trace
    )
    return res


def assemble(results):
    full = np.concatenate(
        [np.asarray(r["out"], dtype=np.float32) for r in results], axis=0
    )  # [OUT_F, T]
    return np.ascontiguousarray(full.T).reshape(2, 1024, OUT_F)


def kernel(x, qweight, scales, bias, lora_A, lora_B):
    in_maps = prep_inputs(x, qweight, scales, bias, lora_A, lora_B)
    res = run(in_maps, trace=False)
    return assemble(res.results)


# revision 10
# speedup vs baseline: 1.1821x; 1.1821x over previous
"""LoftQ linear (4-bit blockwise dequant + linear + LoRA) on 8 trn2 cores.

out = x @ W^T + bias + 2.0 * (x @ A^T) @ B^T
  W[o,i] = (idx[o,i] * 2/15 - 1) * scales[o, i//64]   (idx = 4-bit nibbles)

Sharding: column-parallel — qweight/scales/bias/lora_B sharded along
out_features (4096 -> 512 per core); x and lora_A replicated; outputs
concatenated on host.

Device kernel (per core), all layouts prepared host-side:
  - contraction axis i is permuted to i' = [even i, odd i] so the nibble
    unpack of host-pre-transposed packed bytes lands in contiguous
    partition-tile halves (no on-chip transposes at all).
  - inputs are host-packed into [128, nblk, 512] form so each tensor loads
    with one (or few) large dma_start; DMA work is split across the sync
    HWDGE ring (weights), gpsimd SWDGE (x), and scalar HWDGE (outputs).
  - dequant: bitwise unpack (DVE) -> affine c*v-1 (ScalarE, fp16)
    -> *scale fp16 (DVE) -> + (2BA)^T bf16 (DVE; rank-16 lora product is
    host-precomputed weight preprocessing)
  - main: 512 bf16 matmuls [K=128,M=128,N=512], psum accumulate over i',
    bias added in the psum->sbuf copy (ScalarE), out dma on scalar ring.
"""

import numpy as np
import ml_dtypes

OUT_F = 4096
IN_F = 4096
T = 2048  # 2*1024 tokens
R = 16
NCORES = 8
O_SH = OUT_F // NCORES  # 512
IPH = IN_F // 2  # 2048 packed byte-rows
C16 = 2.0 / 15.0
NQ = IPH // 128  # 16 packed tiles
NI = IN_F // 128  # 32 i' chunks
NO = O_SH // 128  # 4 o tiles
NT = T // 512  # 4 t chunks
NBA = 4  # ba DMA chunks

BF16 = ml_dtypes.bfloat16
FP16 = np.float16

_cached = {}


def _build_nc():
    import concourse.bacc as bacc
    import concourse.mybir as mybir
    from concourse.tile import TileContext

    f32 = mybir.dt.float32
    bf16 = mybir.dt.bfloat16
    fp16 = mybir.dt.float16
    u8 = mybir.dt.uint8
    AF = mybir.ActivationFunctionType
    OP = mybir.AluOpType

    nc = bacc.Bacc("TRN2", target_bir_lowering=False)

    xt = nc.dram_tensor("xt", [128, NT, NI, 512], bf16, kind="ExternalInput")
    qwt = nc.dram_tensor("qwt", [128, NQ, O_SH], u8, kind="ExternalInput")
    st = nc.dram_tensor("st", [128, NQ, O_SH], fp16, kind="ExternalInput")
    ba = nc.dram_tensor("ba", [128, NQ, 2 * O_SH], bf16, kind="ExternalInput")
    bias = nc.dram_tensor("bias", [O_SH, 1], f32, kind="ExternalInput")
    out = nc.dram_tensor("out", [O_SH, T], f32, kind="ExternalOutput")

    with TileContext(nc) as tc:
        with (
            tc.tile_pool(name="w", bufs=1) as wpool,
            tc.tile_pool(name="x", bufs=2) as xpool,
            tc.tile_pool(name="cst", bufs=1) as cpool,
            tc.tile_pool(name="dq", bufs=2) as dqpool,
            tc.tile_pool(name="outp", bufs=3) as opool,
            tc.tile_pool(name="ps", bufs=3, space="PSUM") as pspool,
        ):
            # big resident inputs — one large DMA each on the sync ring
            qb = cpool.tile([128, NQ, O_SH], u8, tag="qb", name="qb")
            nc.sync.dma_start(out=qb[:], in_=qwt[:])
            stb = cpool.tile([128, NQ, O_SH], fp16, tag="stb", name="stb")
            nc.sync.dma_start(out=stb[:], in_=st[:])
            bab = []
            nba = NQ // NBA  # pair-tiles per ba chunk
            for m in range(NBA):
                bt_m = cpool.tile(
                    [128, nba, 2 * O_SH], bf16, tag=f"ba{m}", name=f"bab{m}"
                )
                nc.scalar.dma_start(out=bt_m[:], in_=ba[:, m * nba : (m + 1) * nba, :])
                bab.append(bt_m)

            def bab_sl(k):
                return bab[k // nba][:, k % nba, :]
            bias_sb = []
            for ot in range(NO):
                btile = cpool.tile([128, 1], f32, tag=f"bias{ot}", name=f"biassb{ot}")
                nc.sync.dma_start(out=btile[:], in_=bias[ot * 128 : (ot + 1) * 128, :])
                bias_sb.append(btile)

            # persistent dequantized weight tiles, PAIRED: Wp[k] [128, 1024]
            # cols 0:512 = W[k] (lo nibbles), cols 512:1024 = W[NQ+k] (hi)
            Wp = [
                wpool.tile([128, 2 * O_SH], bf16, tag=f"w{k}", name=f"wt{k}")
                for k in range(NQ)
            ]

            # first x chunk early, split across the scalar + gpsimd rings so
            # PE can start as soon as the first W pairs appear
            xcs = {}
            xcs[0] = xpool.tile([128, NI, 512], bf16, tag="xc", name="xc0")
            nc.scalar.dma_start(out=xcs[0][:, : NI // 2, :], in_=xt[:, 0, : NI // 2])
            nc.gpsimd.dma_start(out=xcs[0][:, NI // 2 :, :], in_=xt[:, 0, NI // 2 :])

            # dequant: packed tile k -> Wp[k]; one op per stage per pair
            for k in range(NQ):
                lh = dqpool.tile([128, 2 * O_SH], u8, tag="lh", name=f"lh{k}")
                nc.vector.tensor_scalar(
                    lh[:, :O_SH], qb[:, k, :], 15, None, OP.bitwise_and
                )
                nc.vector.tensor_scalar(
                    lh[:, O_SH:], qb[:, k, :], 4, 15,
                    OP.logical_shift_right, OP.bitwise_and,
                )
                up = dqpool.tile([128, 2 * O_SH], fp16, tag="up", name=f"up{k}")
                nc.scalar.activation(up[:], lh[:], AF.Copy, bias=-1.0, scale=C16)
                nc.vector.tensor_tensor(
                    Wp[k][:],
                    up[:],
                    stb[:, k, None, :].to_broadcast([128, 2, O_SH]),
                    OP.mult,
                )
                nc.vector.tensor_tensor(Wp[k][:], Wp[k][:], bab_sl(k), OP.add)

            # main matmul, accumulation in W-pair production order
            for tcn in range(NT):
                if tcn not in xcs:
                    xcs[tcn] = xpool.tile(
                        [128, NI, 512], bf16, tag="xc", name=f"xc{tcn}"
                    )
                    nc.gpsimd.dma_start(out=xcs[tcn][:], in_=xt[:, tcn])
                xc = xcs[tcn]
                for ot in range(NO):
                    p = pspool.tile([128, 512], f32, tag="mm", name=f"p{tcn}_{ot}")
                    n = 0
                    for k in range(NQ):
                        for half in range(2):
                            ic = k + half * NQ
                            nc.tensor.matmul(
                                p[:],
                                Wp[k][
                                    :,
                                    half * O_SH + ot * 128 : half * O_SH + (ot + 1) * 128,
                                ],
                                xc[:, ic, :],
                                start=(n == 0),
                                stop=(n == NI - 1),
                            )
                            n += 1
                    o_sb = opool.tile([128, 512], f32, tag="osb", name=f"osb{tcn}_{ot}")
                    nc.scalar.activation(
                        o_sb[:], p[:], AF.Identity, bias=bias_sb[ot][:], scale=1.0
                    )
                    nc.scalar.dma_start(
                        out=out[ot * 128 : (ot + 1) * 128, tcn * 512 : (tcn + 1) * 512],
                        in_=o_sb[:],
                    )
    nc.compile()
    return nc


def _pack_rows(a, nblk):
    """[nblk*128, F] -> [128, nblk, F] with blk j, partition p = row j*128+p."""
    f = a.shape[1]
    return np.ascontiguousarray(a.reshape(nblk, 128, f).transpose(1, 0, 2))


def prep_inputs(x, qweight, scales, bias, lora_A, lora_B):
    """Host-side layout prep + sharding. Returns per-core input maps."""
    x2d = np.ascontiguousarray(x.reshape(T, IN_F))
    xt = x2d.T  # [IN_F, T]
    # i' permutation: even original i first, then odd
    xp = np.concatenate([xt[0::2], xt[1::2]], axis=0)
    xb = _pack_rows(xp, NI)  # [128, NI, T]
    xb = np.ascontiguousarray(
        xb.reshape(128, NI, NT, 512).transpose(0, 2, 1, 3)
    ).astype(BF16)  # [128, NT, NI, 512]

    ap = np.ascontiguousarray(
        np.concatenate([lora_A[:, 0::2], lora_A[:, 1::2]], axis=1)
    ).astype(np.float32)  # [R, IN_F] permuted

    qw2 = qweight.reshape(OUT_F, IPH)  # byte (o, ip) holds i=2ip (lo), 2ip+1 (hi)
    sc2 = scales.reshape(OUT_F, IN_F // 64)

    in_maps = []
    for c in range(NCORES):
        o0, o1 = c * O_SH, (c + 1) * O_SH
        qwt_c = _pack_rows(qw2[o0:o1].T, NQ).astype(np.uint8)  # [128, NQ, O_SH]
        # scale for (ip, o) = scales[o, ip//32] (same for lo and hi nibble)
        st_c = _pack_rows(np.repeat(sc2[o0:o1].T, 32, axis=0), NQ).astype(FP16)
        ba3 = _pack_rows(
            (ap.T @ (2.0 * lora_B[o0:o1].T)).astype(np.float32), NI
        )  # [128, NI, O_SH]
        ba_c = np.ascontiguousarray(
            np.concatenate([ba3[:, :NQ, :], ba3[:, NQ:, :]], axis=2)
        ).astype(BF16)  # [128, NQ, 2*O_SH] pair layout
        bias_c = np.ascontiguousarray(bias[o0:o1].reshape(O_SH, 1)).astype(np.float32)
        in_maps.append(
            {"xt": xb, "qwt": qwt_c, "st": st_c, "ba": ba_c, "bias": bias_c}
        )
    return in_maps


def run(in_maps, trace=False):
    from concourse import bass_utils

    if "nc" not in _cached:
        _cached["nc"] = _build_nc()
    res = bass_utils.run_bass_kernel_spmd(
        _cached["nc"], in_maps, list(range(NCORES)), trace=trace
    )
    return res


def assemble(results):
    full = np.concatenate(
        [np.asarray(r["out"], dtype=np.float32) for r in results], axis=0
    )  # [OUT_F, T]
    return np.ascontiguousarray(full.T).reshape(2, 1024, OUT_F)


def kernel(x, qweight, scales, bias, lora_A, lora_B):
    in_maps = prep_inputs(x, qweight, scales, bias, lora_A, lora_B)
    res = run(in_maps, trace=False)
    return assemble(res.results)


# revision 12
# speedup vs baseline: 1.3523x; 1.1440x over previous
"""LoftQ linear (4-bit blockwise dequant + linear + LoRA) on 8 trn2 cores.

out = x @ W^T + bias + 2.0 * (x @ A^T) @ B^T
  W[o,i] = (idx[o,i] * 2/15 - 1) * scales[o, i//64]   (idx = 4-bit nibbles)

Sharding: column-parallel — qweight/scales/bias/lora_B sharded along
out_features (4096 -> 512 per core); x and lora_A replicated; outputs
concatenated on host.

Device kernel (per core), all layouts prepared host-side:
  - contraction axis i is permuted to i' = [even i, odd i] so the nibble
    unpack of host-pre-transposed packed bytes lands in contiguous
    partition-tile halves (no on-chip transposes at all).
  - inputs are host-packed into [128, nblk, 512] form so each tensor loads
    with one (or few) large dma_start; DMA work is split across the sync
    HWDGE ring (weights), gpsimd SWDGE (x), and scalar HWDGE (outputs).
  - dequant: bitwise unpack (DVE) -> affine c*v-1 (ScalarE, fp16)
    -> *scale fp16 (DVE) -> + (2BA)^T bf16 (DVE; rank-16 lora product is
    host-precomputed weight preprocessing)
  - main: 512 bf16 matmuls [K=128,M=128,N=512], psum accumulate over i',
    bias added in the psum->sbuf copy (ScalarE), out dma on scalar ring.
"""

import numpy as np
import ml_dtypes

OUT_F = 4096
IN_F = 4096
T = 2048  # 2*1024 tokens
R = 16
NCORES = 8
O_SH = OUT_F // NCORES  # 512
IPH = IN_F // 2  # 2048 packed byte-rows
C16 = 2.0 / 15.0
NQ = IPH // 128  # 16 packed tiles
NI = IN_F // 128  # 32 i' chunks
NO = O_SH // 128  # 4 o tiles
NT = T // 512  # 4 t chunks
NBA = 4  # ba DMA chunks

BF16 = ml_dtypes.bfloat16
FP16 = np.float16

_cached = {}


def _build_nc():
    import concourse.bacc as bacc
    import concourse.mybir as mybir
    from concourse.tile import TileContext

    f32 = mybir.dt.float32
    bf16 = mybir.dt.bfloat16
    fp16 = mybir.dt.float16
    u8 = mybir.dt.uint8
    AF = mybir.ActivationFunctionType
    OP = mybir.AluOpType

    nc = bacc.Bacc("TRN2", target_bir_lowering=False)

    xt = nc.dram_tensor("xt", [128, NT, NI, 512], bf16, kind="ExternalInput")
    lh = nc.dram_tensor("lh", [128, NQ, 2 * O_SH], u8, kind="ExternalInput")
    st = nc.dram_tensor("st", [128, NQ, O_SH], fp16, kind="ExternalInput")
    ba = nc.dram_tensor("ba", [128, NQ, 2 * O_SH], bf16, kind="ExternalInput")
    bias = nc.dram_tensor("bias", [O_SH, 1], f32, kind="ExternalInput")
    out = nc.dram_tensor("out", [O_SH, T], f32, kind="ExternalOutput")

    with TileContext(nc) as tc:
        with (
            tc.tile_pool(name="w", bufs=1) as wpool,
            tc.tile_pool(name="x", bufs=2) as xpool,
            tc.tile_pool(name="cst", bufs=1) as cpool,
            tc.tile_pool(name="dq", bufs=2) as dqpool,
            tc.tile_pool(name="outp", bufs=3) as opool,
            tc.tile_pool(name="ps", bufs=4, space="PSUM") as pspool,
        ):
            # bias first (tiny), then W-chain + x interleaved on ONE
            # sync-ring FIFO in consumption-priority order
            bias_sb = []
            for ot in range(NO):
                btile = cpool.tile([128, 1], f32, tag=f"bias{ot}", name=f"biassb{ot}")
                nc.sync.dma_start(out=btile[:], in_=bias[ot * 128 : (ot + 1) * 128, :])
                bias_sb.append(btile)

            Wp = [
                wpool.tile([128, 2 * O_SH], bf16, tag=f"w{k}", name=f"wt{k}")
                for k in range(NQ)
            ]
            lhb = cpool.tile([128, NQ, 2 * O_SH], u8, tag="lhb", name="lhb")
            stb = cpool.tile([128, NQ, O_SH], fp16, tag="stb", name="stb")
            bab = cpool.tile([128, NQ, 2 * O_SH], bf16, tag="bab", name="bab")
            xcs = {}
            xcs[0] = xpool.tile([128, NI, 512], bf16, tag="xc", name="xc0")
            NC_ = NQ // NBA  # pair-tiles per chunk (4)
            for m in range(NBA):
                ks = slice(m * NC_, (m + 1) * NC_)
                nc.sync.dma_start(out=lhb[:, ks, :], in_=lh[:, ks, :])
                nc.sync.dma_start(out=stb[:, ks, :], in_=st[:, ks, :])
                nc.sync.dma_start(out=bab[:, ks, :], in_=ba[:, ks, :])
                # x chunk-0 sub-blocks for this round's pairs: pair k uses
                # x blocks k (lo) and NQ+k (hi)
                nc.sync.dma_start(
                    out=xcs[0][:, m * NC_ : (m + 1) * NC_, :],
                    in_=xt[:, 0, m * NC_ : (m + 1) * NC_],
                )
                nc.sync.dma_start(
                    out=xcs[0][:, NQ + m * NC_ : NQ + (m + 1) * NC_, :],
                    in_=xt[:, 0, NQ + m * NC_ : NQ + (m + 1) * NC_],
                )

            # dequant: host-unpacked nibbles -> affine (ScalarE) -> *scale
            # + lora add (DVE); all ops one-per-pair on [128, 1024] tiles
            for k in range(NQ):
                up = dqpool.tile([128, 2 * O_SH], fp16, tag="up", name=f"up{k}")
                nc.scalar.activation(
                    up[:], lhb[:, k, :], AF.Copy, bias=-1.0, scale=C16
                )
                nc.vector.tensor_tensor(
                    Wp[k][:],
                    up[:],
                    stb[:, k, None, :].to_broadcast([128, 2, O_SH]),
                    OP.mult,
                )
                nc.vector.tensor_tensor(Wp[k][:], Wp[k][:], bab[:, k, :], OP.add)

            # main matmul, accumulation in W-pair production order
            for tcn in range(NT):
                if tcn not in xcs:
                    xcs[tcn] = xpool.tile(
                        [128, NI, 512], bf16, tag="xc", name=f"xc{tcn}"
                    )
                    nc.sync.dma_start(out=xcs[tcn][:], in_=xt[:, tcn])
                xc = xcs[tcn]
                for ot in range(NO):
                    p = pspool.tile([128, 512], f32, tag="mm", name=f"p{tcn}_{ot}")
                    n = 0
                    for k in range(NQ):
                        for half in range(2):
                            ic = k + half * NQ
                            nc.tensor.matmul(
                                p[:],
                                Wp[k][
                                    :,
                                    half * O_SH + ot * 128 : half * O_SH + (ot + 1) * 128,
                                ],
                                xc[:, ic, :],
                                start=(n == 0),
                                stop=(n == NI - 1),
                            )
                            n += 1
                    o_sb = opool.tile([128, 512], f32, tag="osb", name=f"osb{tcn}_{ot}")
                    nc.scalar.activation(
                        o_sb[:], p[:], AF.Identity, bias=bias_sb[ot][:], scale=1.0
                    )
                    nc.scalar.dma_start(
                        out=out[ot * 128 : (ot + 1) * 128, tcn * 512 : (tcn + 1) * 512],
                        in_=o_sb[:],
                    )
    nc.compile()
    return nc


def _pack_rows(a, nblk):
    """[nblk*128, F] -> [128, nblk, F] with blk j, partition p = row j*128+p."""
    f = a.shape[1]
    return np.ascontiguousarray(a.reshape(nblk, 128, f).transpose(1, 0, 2))


def prep_inputs(x, qweight, scales, bias, lora_A, lora_B):
    """Host-side layout prep + sharding. Returns per-core input maps."""
    x2d = np.ascontiguousarray(x.reshape(T, IN_F))
    xt = x2d.T  # [IN_F, T]
    # i' permutation: even original i first, then odd
    xp = np.concatenate([xt[0::2], xt[1::2]], axis=0)
    xb = _pack_rows(xp, NI)  # [128, NI, T]
    xb = np.ascontiguousarray(
        xb.reshape(128, NI, NT, 512).transpose(0, 2, 1, 3)
    ).astype(BF16)  # [128, NT, NI, 512]

    ap = np.ascontiguousarray(
        np.concatenate([lora_A[:, 0::2], lora_A[:, 1::2]], axis=1)
    ).astype(np.float32)  # [R, IN_F] permuted

    qw2 = qweight.reshape(OUT_F, IPH)  # byte (o, ip) holds i=2ip (lo), 2ip+1 (hi)
    sc2 = scales.reshape(OUT_F, IN_F // 64)

    in_maps = []
    for c in range(NCORES):
        o0, o1 = c * O_SH, (c + 1) * O_SH
        qp = _pack_rows(qw2[o0:o1].T, NQ)  # [128, NQ, O_SH] packed bytes
        lh_c = np.ascontiguousarray(
            np.concatenate([qp & 15, (qp >> 4) & 15], axis=2)
        ).astype(np.uint8)  # [128, NQ, 2*O_SH] nibbles, pair layout
        # scale for (ip, o) = scales[o, ip//32] (same for lo and hi nibble)
        st_c = _pack_rows(np.repeat(sc2[o0:o1].T, 32, axis=0), NQ).astype(FP16)
        ba3 = _pack_rows(
            (ap.T @ (2.0 * lora_B[o0:o1].T)).astype(np.float32), NI
        )  # [128, NI, O_SH]
        ba_c = np.ascontiguousarray(
            np.concatenate([ba3[:, :NQ, :], ba3[:, NQ:, :]], axis=2)
        ).astype(BF16)  # [128, NQ, 2*O_SH] pair layout
        bias_c = np.ascontiguousarray(bias[o0:o1].reshape(O_SH, 1)).astype(np.float32)
        in_maps.append(
            {"xt": xb, "lh": lh_c, "st": st_c, "ba": ba_c, "bias": bias_c}
        )
    return in_maps


def run(in_maps, trace=False):
    from concourse import bass_utils

    if "nc" not in _cached:
        _cached["nc"] = _build_nc()
    res = bass_utils.run_bass_kernel_spmd(
        _cached["nc"], in_maps, list(range(NCORES)), trace=trace
    )
    return res


def assemble(results):
    full = np.concatenate(
        [np.asarray(r["out"], dtype=np.float32) for r in results], axis=0
    )  # [OUT_F, T]
    return np.ascontiguousarray(full.T).reshape(2, 1024, OUT_F)


def kernel(x, qweight, scales, bias, lora_A, lora_B):
    in_maps = prep_inputs(x, qweight, scales, bias, lora_A, lora_B)
    res = run(in_maps, trace=False)
    return assemble(res.results)


# revision 13
# speedup vs baseline: 1.3958x; 1.0321x over previous
"""LoftQ linear (4-bit blockwise dequant + linear + LoRA) on 8 trn2 cores.

out = x @ W^T + bias + 2.0 * (x @ A^T) @ B^T
  W[o,i] = (idx[o,i] * 2/15 - 1) * scales[o, i//64]   (idx = 4-bit nibbles)

Sharding: column-parallel — qweight/scales/bias/lora_B sharded along
out_features (4096 -> 512 per core); x and lora_A replicated; outputs
concatenated on host.

Device kernel (per core), all layouts prepared host-side:
  - contraction axis i is permuted to i' = [even i, odd i] so the nibble
    unpack of host-pre-transposed packed bytes lands in contiguous
    partition-tile halves (no on-chip transposes at all).
  - inputs are host-packed into [128, nblk, 512] form so each tensor loads
    with one (or few) large dma_start; DMA work is split across the sync
    HWDGE ring (weights), gpsimd SWDGE (x), and scalar HWDGE (outputs).
  - dequant: bitwise unpack (DVE) -> affine c*v-1 (ScalarE, fp16)
    -> *scale fp16 (DVE) -> + (2BA)^T bf16 (DVE; rank-16 lora product is
    host-precomputed weight preprocessing)
  - main: 512 bf16 matmuls [K=128,M=128,N=512], psum accumulate over i',
    bias added in the psum->sbuf copy (ScalarE), out dma on scalar ring.
"""

import numpy as np
import ml_dtypes

OUT_F = 4096
IN_F = 4096
T = 2048  # 2*1024 tokens
R = 16
NCORES = 8
O_SH = OUT_F // NCORES  # 512
IPH = IN_F // 2  # 2048 packed byte-rows
C16 = 2.0 / 15.0
NQ = IPH // 128  # 16 packed tiles
NI = IN_F // 128  # 32 i' chunks
NO = O_SH // 128  # 4 o tiles
NT = T // 512  # 4 t chunks
NBA = 4  # ba DMA chunks

BF16 = ml_dtypes.bfloat16
FP16 = np.float16

_cached = {}


def _build_nc():
    import concourse.bacc as bacc
    import concourse.mybir as mybir
    from concourse.tile import TileContext

    f32 = mybir.dt.float32
    bf16 = mybir.dt.bfloat16
    fp16 = mybir.dt.float16
    u8 = mybir.dt.uint8
    AF = mybir.ActivationFunctionType
    OP = mybir.AluOpType

    nc = bacc.Bacc("TRN2", target_bir_lowering=False)

    xt = nc.dram_tensor("xt", [128, NT, NI, 512], bf16, kind="ExternalInput")
    lh = nc.dram_tensor("lh", [128, NQ, 2 * O_SH], u8, kind="ExternalInput")
    st = nc.dram_tensor("st", [128, NQ, O_SH], fp16, kind="ExternalInput")
    ba = nc.dram_tensor("ba", [128, NQ, 2 * O_SH], bf16, kind="ExternalInput")
    bias = nc.dram_tensor("bias", [O_SH, 1], f32, kind="ExternalInput")
    out = nc.dram_tensor("out", [O_SH, T], f32, kind="ExternalOutput")

    with TileContext(nc) as tc:
        with (
            tc.tile_pool(name="w", bufs=1) as wpool,
            tc.tile_pool(name="x", bufs=2) as xpool,
            tc.tile_pool(name="cst", bufs=1) as cpool,
            tc.tile_pool(name="dq", bufs=2) as dqpool,
            tc.tile_pool(name="outp", bufs=3) as opool,
            tc.tile_pool(name="ps", bufs=6, space="PSUM") as pspool,
        ):
            # bias first (tiny), then W-chain + x interleaved on ONE
            # sync-ring FIFO in consumption-priority order
            bias_sb = []
            for ot in range(NO):
                btile = cpool.tile([128, 1], f32, tag=f"bias{ot}", name=f"biassb{ot}")
                nc.sync.dma_start(out=btile[:], in_=bias[ot * 128 : (ot + 1) * 128, :])
                bias_sb.append(btile)

            Wp = [
                wpool.tile([128, 2 * O_SH], bf16, tag=f"w{k}", name=f"wt{k}")
                for k in range(NQ)
            ]
            lhb = []
            stbs = []
            babs = []
            xc0t = []
            NC_ = NQ // NBA  # pair-tiles per chunk (4)
            for m in range(NBA):
                lhb.append(
                    cpool.tile([128, NC_, 2 * O_SH], u8, tag=f"lhb{m}", name=f"lhb{m}")
                )
                stbs.append(
                    cpool.tile([128, NC_, O_SH], fp16, tag=f"stb{m}", name=f"stb{m}")
                )
                babs.append(
                    cpool.tile([128, NC_, 2 * O_SH], bf16, tag=f"bab{m}", name=f"bab{m}")
                )
                xc0t.append(
                    cpool.tile([128, 2 * NC_, 512], bf16, tag=f"xc0t{m}", name=f"xc0t{m}")
                )
            xcs = {}
            for m in range(NBA):
                ks = slice(m * NC_, (m + 1) * NC_)
                nc.sync.dma_start(out=lhb[m][:], in_=lh[:, ks, :])
                nc.sync.dma_start(out=stbs[m][:], in_=st[:, ks, :])
                nc.sync.dma_start(out=babs[m][:], in_=ba[:, ks, :])
                # x chunk-0 blocks for this round's pairs: pair k uses x
                # blocks k (lo) and NQ+k (hi)
                nc.sync.dma_start(
                    out=xc0t[m][:, :NC_, :],
                    in_=xt[:, 0, m * NC_ : (m + 1) * NC_],
                )
                nc.sync.dma_start(
                    out=xc0t[m][:, NC_:, :],
                    in_=xt[:, 0, NQ + m * NC_ : NQ + (m + 1) * NC_],
                )

            def xc0_sl(ic):
                # x chunk-0 block ic -> (round tile, sub-index)
                if ic < NQ:
                    return xc0t[ic // NC_][:, ic % NC_, :]
                icl = ic - NQ
                return xc0t[icl // NC_][:, NC_ + icl % NC_, :]

            # dequant: host-unpacked nibbles -> affine (ScalarE) -> *scale
            # + lora add (DVE); all ops one-per-pair on [128, 1024] tiles
            for k in range(NQ):
                m, j = k // NC_, k % NC_
                up = dqpool.tile([128, 2 * O_SH], fp16, tag="up", name=f"up{k}")
                nc.scalar.activation(
                    up[:], lhb[m][:, j, :], AF.Copy, bias=-1.0, scale=C16
                )
                nc.vector.tensor_tensor(
                    Wp[k][:],
                    up[:],
                    stbs[m][:, j, None, :].to_broadcast([128, 2, O_SH]),
                    OP.mult,
                )
                nc.vector.tensor_tensor(Wp[k][:], Wp[k][:], babs[m][:, j, :], OP.add)

            # main matmul, accumulation in W-pair production order
            for tcn in range(NT):
                if tcn > 0:
                    xcs[tcn] = xpool.tile(
                        [128, NI, 512], bf16, tag="xc", name=f"xc{tcn}"
                    )
                    nc.sync.dma_start(out=xcs[tcn][:], in_=xt[:, tcn])
                for ot in range(NO):
                    p = pspool.tile([128, 512], f32, tag="mm", name=f"p{tcn}_{ot}")
                    n = 0
                    for k in range(NQ):
                        for half in range(2):
                            ic = k + half * NQ
                            xs = (
                                xc0_sl(ic) if tcn == 0 else xcs[tcn][:, ic, :]
                            )
                            nc.tensor.matmul(
                                p[:],
                                Wp[k][
                                    :,
                                    half * O_SH + ot * 128 : half * O_SH + (ot + 1) * 128,
                                ],
                                xs,
                                start=(n == 0),
                                stop=(n == NI - 1),
                            )
                            n += 1
                    o_sb = opool.tile([128, 512], f32, tag="osb", name=f"osb{tcn}_{ot}")
                    nc.vector.tensor_scalar(
                        o_sb[:], p[:], bias_sb[ot][:], None, OP.add
                    )
                    nc.scalar.dma_start(
                        out=out[ot * 128 : (ot + 1) * 128, tcn * 512 : (tcn + 1) * 512],
                        in_=o_sb[:],
                    )
    nc.compile()
    return nc


def _pack_rows(a, nblk):
    """[nblk*128, F] -> [128, nblk, F] with blk j, partition p = row j*128+p."""
    f = a.shape[1]
    return np.ascontiguousarray(a.reshape(nblk, 128, f).transpose(1, 0, 2))


def prep_inputs(x, qweight, scales, bias, lora_A, lora_B):
    """Host-side layout prep + sharding. Returns per-core input maps."""
    x2d = np.ascontiguousarray(x.reshape(T, IN_F))
    xt = x2d.T  # [IN_F, T]
    # i' permutation: even original i first, then odd
    xp = np.concatenate([xt[0::2], xt[1::2]], axis=0)
    xb = _pack_rows(xp, NI)  # [128, NI, T]
    xb = np.ascontiguousarray(
        xb.reshape(128, NI, NT, 512).transpose(0, 2, 1, 3)
    ).astype(BF16)  # [128, NT, NI, 512]

    ap = np.ascontiguousarray(
        np.concatenate([lora_A[:, 0::2], lora_A[:, 1::2]], axis=1)
    ).astype(np.float32)  # [R, IN_F] permuted

    qw2 = qweight.reshape(OUT_F, IPH)  # byte (o, ip) holds i=2ip (lo), 2ip+1 (hi)
    sc2 = scales.reshape(OUT_F, IN_F // 64)

    in_maps = []
    for c in range(NCORES):
        o0, o1 = c * O_SH, (c + 1) * O_SH
        qp = _pack_rows(qw2[o0:o1].T, NQ)  # [128, NQ, O_SH] packed bytes
        lh_c = np.ascontiguousarray(
            np.concatenate([qp & 15, (qp >> 4) & 15], axis=2)
        ).astype(np.uint8)  # [128, NQ, 2*O_SH] nibbles, pair layout
        # scale for (ip, o) = scales[o, ip//32] (same for lo and hi nibble)
        st_c = _pack_rows(np.repeat(sc2[o0:o1].T, 32, axis=0), NQ).astype(FP16)
        ba3 = _pack_rows(
            (ap.T @ (2.0 * lora_B[o0:o1].T)).astype(np.float32), NI
        )  # [128, NI, O_SH]
        ba_c = np.ascontiguousarray(
            np.concatenate([ba3[:, :NQ, :], ba3[:, NQ:, :]], axis=2)
        ).astype(BF16)  # [128, NQ, 2*O_SH] pair layout
        bias_c = np.ascontiguousarray(bias[o0:o1].reshape(O_SH, 1)).astype(np.float32)
        in_maps.append(
            {"xt": xb, "lh": lh_c, "st": st_c, "ba": ba_c, "bias": bias_c}
        )
    return in_maps


def run(in_maps, trace=False):
    from concourse import bass_utils

    if "nc" not in _cached:
        _cached["nc"] = _build_nc()
    res = bass_utils.run_bass_kernel_spmd(
        _cached["nc"], in_maps, list(range(NCORES)), trace=trace
    )
    return res


def assemble(results):
    full = np.concatenate(
        [np.asarray(r["out"], dtype=np.float32) for r in results], axis=0
    )  # [OUT_F, T]
    return np.ascontiguousarray(full.T).reshape(2, 1024, OUT_F)


def kernel(x, qweight, scales, bias, lora_A, lora_B):
    in_maps = prep_inputs(x, qweight, scales, bias, lora_A, lora_B)
    res = run(in_maps, trace=False)
    return assemble(res.results)
